# revision 1
# baseline (speedup 1.0000x reference)
"""Trainium2 Bass kernel for nn_Deep_Mem_RelativeLocs_ProjectedLowerDim.

out = mem + counts.reshape(IDX_DIMS + (1,1,1)) where counts is an 80000-bin
histogram of hashed rel_vec rows.

Strategy (8 cores, data-parallel over rel_vec rows):
 - Host: transpose rel_vec shard, split into bf16 hi/lo planes (same bytes as
   f32), pack per-super-chunk for efficient DMA.  A -0.5 bias row is folded
   into the hash matmul so that the round-to-nearest f32->i32 cast gives floor.
 - PE: h^T = w.T @ relT via 4 bf16 matmuls (hi*w_hi+mid accum in PSUM[14,:]),
   transpose h back to rows-on-partitions.
 - DVE: h=main+mid, clamp, strides-dot -> flat bucket id; hi=flat//625,
   lo=flat%625; one-hot via is_equal against f16 iotas.
 - PE: counts[hi,lo] += A^T B per 128-row chunk accumulated in PSUM [128,626].
 - ReduceScatter counts across 8 cores; each core adds its 10000-bucket slice
   broadcast over the trailing 200-slab and writes its 8MB output shard.
"""
import numpy as np
import ml_dtypes

# ---- problem constants (hardcoded; must match the harness problem) ----
N_ROWS = 415744
RV_W = 241
N_CORES = 8
ROWS_PER_CORE = N_ROWS // N_CORES            # 51968
CHUNK = 128
N_CHUNKS = ROWS_PER_CORE // CHUNK            # 406
SUP_CHUNKS = 16                              # chunks per super-chunk (DMA unit)
N_SUP = (N_CHUNKS + SUP_CHUNKS - 1) // SUP_CHUNKS   # 26 (last has 6)
IDX_DIMS = (2, 10, 10, 2, 10, 10, 2)
BOUNDS = [d - 1 for d in IDX_DIMS]
STRIDES = [40000, 4000, 400, 200, 20, 2, 1]
N_FLAT = 80000
HI = 128
LO = 625
LOP = 626                                    # padded even width
TRAIL = 200
BPC = N_FLAT // N_CORES                      # 10000 buckets per core
MEM_SIZE = (2, 10, 10, 2, 10, 10, 2, 10, 10, 2)

K0 = 128                                     # feature K-tile sizes
K1 = RV_W - K0                               # 113
SECT = SUP_CHUNKS * CHUNK                    # 2048 rows per super
PL_W = 4 * SECT                              # packed plane width per super

_nc_cache = {}


def _build_nc():
    from contextlib import ExitStack
    import concourse.bacc as bacc
    import concourse.tile as tile
    import concourse.mybir as mybir

    f32 = mybir.dt.float32
    f16 = mybir.dt.float16
    bf16 = mybir.dt.bfloat16
    i32 = mybir.dt.int32
    Alu = mybir.AluOpType

    nc = bacc.Bacc("TRN2", target_bir_lowering=False, debug=False,
                   enable_asserts=False, num_devices=N_CORES)

    planes = nc.dram_tensor("planes", [N_SUP, 128, PL_W], bf16, kind="ExternalInput")
    hwTp = nc.dram_tensor("hwTp", [128, 28], bf16, kind="ExternalInput")
    iota_h = nc.dram_tensor("iota_h", [128, HI], f16, kind="ExternalInput")
    iota_l = nc.dram_tensor("iota_l", [128, LOP], f16, kind="ExternalInput")
    ident = nc.dram_tensor("ident", [128, 16], f32, kind="ExternalInput")
    bounds = nc.dram_tensor("bounds", [128, SUP_CHUNKS * 7], f32, kind="ExternalInput")
    strides = nc.dram_tensor("strides", [128, SUP_CHUNKS * 7], f32, kind="ExternalInput")
    memsh = nc.dram_tensor("memsh", [BPC, TRAIL], f32, kind="ExternalInput")
    out = nc.dram_tensor("out", [BPC, TRAIL], f32, kind="ExternalOutput")

    with tile.TileContext(nc) as tc, ExitStack() as ctx:
        cpool = ctx.enter_context(tc.tile_pool(name="consts", bufs=1))
        relpool = ctx.enter_context(tc.tile_pool(name="rel", bufs=2))
        hsbp = ctx.enter_context(tc.tile_pool(name="hsb", bufs=3))
        hTsbp = ctx.enter_context(tc.tile_pool(name="hTsb", bufs=2))
        arith = ctx.enter_context(tc.tile_pool(name="arith", bufs=2))
        cmpp = ctx.enter_context(tc.tile_pool(name="cmp", bufs=3))
        tailp = ctx.enter_context(tc.tile_pool(name="tail", bufs=1))
        memp = ctx.enter_context(tc.tile_pool(name="mem", bufs=1))
        hps = ctx.enter_context(tc.tile_pool(name="hps", bufs=2, space="PSUM"))
        hTps = ctx.enter_context(tc.tile_pool(name="hTps", bufs=2, space="PSUM"))
        ctps = ctx.enter_context(tc.tile_pool(name="ctps", bufs=1, space="PSUM"))
        dram = ctx.enter_context(tc.tile_pool(name="dram", bufs=1, space="DRAM"))

        # ---- constants
        hwTp_sb = cpool.tile([128, 28], bf16)
        nc.sync.dma_start(hwTp_sb[:], hwTp[:])
        iota_h_sb = cpool.tile([128, HI], f16)
        nc.sync.dma_start(iota_h_sb[:], iota_h[:])
        iota_l_sb = cpool.tile([128, LOP], f16)
        nc.sync.dma_start(iota_l_sb[:], iota_l[:])
        id_sb = cpool.tile([128, 16], f32)
        nc.sync.dma_start(id_sb[:], ident[:])
        bounds_sb = cpool.tile([128, SUP_CHUNKS * 7], f32)
        nc.sync.dma_start(bounds_sb[:], bounds[:])
        strides_sb = cpool.tile([128, SUP_CHUNKS * 7], f32)
        nc.sync.dma_start(strides_sb[:], strides[:])

        mem_sb = memp.tile([125, 16000], f32)
        memr = memsh[:].rearrange("(p b) t -> p (b t)", p=125)

        counts_ps = ctps.tile([128, LOP], f32)

        chunk_idx = 0
        for s in range(N_SUP):
            S = min(SUP_CHUNKS, N_CHUNKS - s * SUP_CHUNKS)
            pl = relpool.tile([128, PL_W], bf16, tag="pl")
            nc.sync.dma_start(pl[:], planes[s, :, :])
            # sections within pl: 0:hi_k0 1:lo_k0 2:hi_k1 3:lo_k1
            hi_k0 = pl[:, 0 * SECT:0 * SECT + S * CHUNK]
            lo_k0 = pl[:, 1 * SECT:1 * SECT + S * CHUNK]
            hi_k1 = pl[0:K1 + 1, 2 * SECT:2 * SECT + S * CHUNK]   # +1: ones bias row
            lo_k1 = pl[0:K1, 3 * SECT:3 * SECT + S * CHUNK]

            # mem prefetch spread over mid supers (4 x 2MB)
            if 18 <= s <= 21:
                q = s - 18
                nc.sync.dma_start(mem_sb[:, q * 4000:(q + 1) * 4000],
                                  memr[:, q * 4000:(q + 1) * 4000])

            hT_ps = hTps.tile([128, SUP_CHUNKS * 14], f32, tag="hTps")
            for g in range(S // 2):
                cols = slice(g * 256, (g + 1) * 256)
                h_ps = hps.tile([14, 256], f32, tag="hps")
                nc.tensor.matmul(h_ps[:], hwTp_sb[:, 0:14], hi_k0[:, cols],
                                 start=True, stop=False)
                nc.tensor.matmul(h_ps[:], hwTp_sb[0:K1 + 1, 14:28], hi_k1[:, cols],
                                 start=False, stop=False)
                nc.tensor.matmul(h_ps[0:7, :], hwTp_sb[:, 0:7], lo_k0[:, cols],
                                 start=False, stop=False)
                nc.tensor.matmul(h_ps[0:7, :], hwTp_sb[0:K1, 14:21], lo_k1[:, cols],
                                 start=False, stop=True)
                h_sb = hsbp.tile([14, 256], f32, tag="hsb")
                nc.scalar.copy(h_sb[:], h_ps[:])
                for j in range(2):
                    cc = 2 * g + j
                    nc.tensor.transpose(hT_ps[:, cc * 14:(cc + 1) * 14],
                                        h_sb[:, j * 128:(j + 1) * 128],
                                        id_sb[0:14, 0:14])

            hT_sb = hTsbp.tile([128, SUP_CHUNKS * 14], f32, tag="hTsb")
            nc.scalar.copy(hT_sb[:, 0:S * 14], hT_ps[:, 0:S * 14])

            # DVE arithmetic (bias -0.5 already folded into h)
            hfloor = arith.tile([128, SUP_CHUNKS * 7], i32, tag="hfloor")
            main_ap = hT_sb[:, 0:S * 14].rearrange("p (c t) -> p c t", t=14)[:, :, 0:7]
            mid_ap = hT_sb[:, 0:S * 14].rearrange("p (c t) -> p c t", t=14)[:, :, 7:14]
            hf_ap = hfloor[:, 0:S * 7].rearrange("p (c t) -> p c t", t=7)
            nc.vector.tensor_tensor(hf_ap, main_ap, mid_ap, Alu.add)

            clamped = arith.tile([128, SUP_CHUNKS * 7], f32, tag="clamped")
            nc.vector.tensor_tensor(clamped[:, 0:S * 7], hfloor[:, 0:S * 7],
                                    bounds_sb[:, 0:S * 7], Alu.min)
            nc.vector.tensor_tensor(clamped[:, 0:S * 7], clamped[:, 0:S * 7],
                                    strides_sb[:, 0:S * 7], Alu.mult)
            flat = arith.tile([128, SUP_CHUNKS], f32, tag="flat")
            nc.vector.tensor_reduce(
                flat[:, 0:S],
                clamped[:, 0:S * 7].rearrange("p (c t) -> p c t", t=7),
                mybir.AxisListType.X, Alu.add)

            hi_i = arith.tile([128, SUP_CHUNKS], i32, tag="hi_i")
            nc.vector.tensor_scalar(hi_i[:, 0:S], flat[:, 0:S], 1.0 / 625.0, -0.5,
                                    Alu.mult, Alu.add)
            hi_f = arith.tile([128, SUP_CHUNKS], f32, tag="hi_f")
            nc.vector.tensor_copy(hi_f[:, 0:S], hi_i[:, 0:S])
            neg = arith.tile([128, SUP_CHUNKS], f32, tag="neg")
            nc.vector.tensor_scalar(neg[:, 0:S], hi_i[:, 0:S], -625.0, None, Alu.mult)
            lo_f = arith.tile([128, SUP_CHUNKS], f32, tag="lo_f")
            nc.vector.tensor_tensor(lo_f[:, 0:S], neg[:, 0:S], flat[:, 0:S], Alu.add)

            for j in range(S):
                A = cmpp.tile([128, HI], bf16, tag="A")
                nc.vector.tensor_scalar(A[:], iota_h_sb[:], hi_f[:, j:j + 1], None,
                                        Alu.is_equal)
                B = cmpp.tile([128, LOP], bf16, tag="B")
                nc.vector.tensor_scalar(B[:], iota_l_sb[:], lo_f[:, j:j + 1], None,
                                        Alu.is_equal)
                first = chunk_idx == 0
                last = chunk_idx == N_CHUNKS - 1
                nc.tensor.matmul(counts_ps[:, 0:512], A[:], B[:, 0:512],
                                 start=first, stop=last, skip_group_check=True)
                nc.tensor.matmul(counts_ps[:, 512:LOP], A[:], B[:, 512:LOP],
                                 start=first, stop=last, skip_group_check=True)
                chunk_idx += 1

        # ---- tail: reduce counts across cores, add to mem shard, write out
        counts_sb = tailp.tile([128, LOP], f32)
        nc.vector.tensor_copy(counts_sb[:], counts_ps[:])

        counts_dram = dram.tile([128, LO], f32)
        red_dram = dram.tile([16, LO], f32)
        nc.sync.dma_start(counts_dram[:], counts_sb[:, 0:LO])
        nc.gpsimd.collective_compute(
            "ReduceScatter", Alu.add,
            replica_groups=[list(range(N_CORES))],
            ins=[counts_dram.opt()],
            outs=[red_dram.opt()],
        )
        red_sb = tailp.tile([125, 80], f32)
        nc.sync.dma_start(red_sb[:], red_dram[:].rearrange("a b -> (a b)").rearrange("(p c) -> p c", p=125))

        red_b = red_sb[:].unsqueeze(2).broadcast_to([125, 80, TRAIL])
        mem3 = mem_sb[:].rearrange("p (c t) -> p c t", t=TRAIL)
        nc.vector.tensor_tensor(mem3, mem3, red_b, Alu.add)

        outr = out[:].rearrange("(p b) t -> p (b t)", p=125)
        for q in range(4):
            nc.sync.dma_start(outr[:, q * 4000:(q + 1) * 4000],
                              mem_sb[:, q * 4000:(q + 1) * 4000])

    nc.compile()
    return nc


def _host_prep(rel_vec, hash_w):
    """Build per-core packed bf16 hi/lo planes + constant tensors."""
    bf = ml_dtypes.bfloat16
    consts = {}
    w = hash_w.T.astype(np.float32)                      # [241, 7]
    w_hi = w.astype(bf).astype(np.float32)
    w_mid = (w - w_hi).astype(bf).astype(np.float32)
    hwTp = np.zeros((128, 28), np.float32)
    hwTp[:, 0:7] = w_hi[0:K0]
    hwTp[:, 7:14] = w_mid[0:K0]
    hwTp[0:K1, 14:21] = w_hi[K0:RV_W]
    hwTp[0:K1, 21:28] = w_mid[K0:RV_W]
    hwTp[K1, 14:21] = -0.5                               # floor bias row
    consts["hwTp"] = hwTp.astype(bf)

    consts["iota_h"] = np.broadcast_to(
        np.arange(HI, dtype=np.float16)[None, :], (128, HI)).copy()
    il = np.arange(LOP, dtype=np.float16)
    il[LO] = 10000.0                                     # pad col never matches
    consts["iota_l"] = np.broadcast_to(il[None, :], (128, LOP)).copy()
    ident = np.zeros((128, 16), np.float32)
    ident[0:14, 0:14] = np.eye(14, dtype=np.float32)
    consts["ident"] = ident
    consts["bounds"] = np.broadcast_to(
        np.tile(np.array(BOUNDS, np.float32), SUP_CHUNKS)[None, :],
        (128, SUP_CHUNKS * 7)).copy()
    consts["strides"] = np.broadcast_to(
        np.tile(np.array(STRIDES, np.float32), SUP_CHUNKS)[None, :],
        (128, SUP_CHUNKS * 7)).copy()

    # per-core planes
    pad_rows = N_SUP * SECT - ROWS_PER_CORE              # 1280
    planes_all = []
    for c in range(N_CORES):
        shard = rel_vec[c * ROWS_PER_CORE:(c + 1) * ROWS_PER_CORE]
        if pad_rows:
            shard = np.concatenate(
                [shard, np.zeros((pad_rows, RV_W), np.float32)], axis=0)
        R = shard.reshape(N_SUP, SECT, RV_W)
        hi = R.astype(bf)
        lo = (R - hi.astype(np.float32)).astype(bf)
        hiT = np.ascontiguousarray(hi.transpose(0, 2, 1))    # [S, 241, 2048]
        loT = np.ascontiguousarray(lo.transpose(0, 2, 1))
        pk = np.zeros((N_SUP, 128, PL_W), bf)
        pk[:, :, 0 * SECT:1 * SECT] = hiT[:, 0:K0]
        pk[:, :, 1 * SECT:2 * SECT] = loT[:, 0:K0]
        pk[:, 0:K1, 2 * SECT:3 * SECT] = hiT[:, K0:RV_W]
        pk[:, K1, 2 * SECT:3 * SECT] = bf(1.0)               # ones bias row
        pk[:, 0:K1, 3 * SECT:4 * SECT] = loT[:, K0:RV_W]
        planes_all.append(pk)
    return consts, planes_all


def kernel(rel_vec, hash_w, mem):
    from concourse import bass_utils

    rel_vec = np.asarray(rel_vec, np.float32)
    hash_w = np.asarray(hash_w, np.float32)
    mem = np.asarray(mem, np.float32)

    if "nc" not in _nc_cache:
        _nc_cache["nc"] = _build_nc()
    nc = _nc_cache["nc"]

    consts, planes_all = _host_prep(rel_vec, hash_w)
    mem_flat = mem.reshape(N_FLAT, TRAIL)

    in_maps = []
    for c in range(N_CORES):
        m = dict(consts)
        m["planes"] = planes_all[c]
        m["memsh"] = np.ascontiguousarray(mem_flat[c * BPC:(c + 1) * BPC])
        in_maps.append(m)

    res = bass_utils.run_bass_kernel_spmd(nc, in_maps, core_ids=list(range(N_CORES)))
    out = np.concatenate([r["out"] for r in res.results], axis=0)
    return out.reshape(MEM_SIZE)

